# revision 16
# baseline (speedup 1.0000x reference)
"""Trainium2 Bass kernel for nn_BoundaryLoss: mean(|softmax(pred) * SDF(onehot(target))|).

Strategy (8 NeuronCores, SPMD, one (b, c) pair per core):
  - Exact 3D squared EDT of the class-c mask (pos) and complement (neg) via
    truncated-shift separable min-plus with certified radius S (host proves the
    truncation exact: if the S-truncated result's max squared distance M satisfies
    floor(sqrt(M)) <= S, every voxel's optimal seed lies in the [-S,S]^3 box).
  - All EDT arithmetic in bf16 (exact: values are small integers or INF=16384,
    which only meets min/add ops that keep it >= any real distance).
  - Layout [96 partitions, 48*PITCH free]: rows [0,48) pos | [48,96) neg;
    free = (h, w padded to PITCH with INF). The D (partition) pass consumes
    host-prelaid shifted+biased f0 images streamed from DRAM, so it is pure
    2x-mode tensor_tensor mins with no partition-offset DMA. H and W passes
    build one biased copy per radius with a 4x tensor_scalar (written shifted
    for odd W radii so every min stays 4B-aligned / 2x-mode) and min with
    plain tensor_tensor (the fused scalar_tensor_tensor only has a 1x uop).
  - softmax: ACT exp on host-prelaid (class,d)-partition layouts; denominator
    via PE matmul with a class-sum stationary; reciprocal via bf16 bit-hack +
    one Newton step on DVE (bass bans the ACT Reciprocal table; DVE reciprocal
    is ~6 cyc/elem). pos+neg pair-sum via PE matmul, sqrt on ACT from PSUM,
    final row-reduce on ACT via activation accum_out (scale=-1 absorbs the
    Newton sign).
  - Host shards inputs, sums the 8x48 f32 partials, applies the has_pos gate
    and the 1/(B*C*D*H*W) factor.
"""

import os
import sys

import numpy as np

B, C, DD, HH, WW = 2, 4, 48, 48, 48
NVOX = DD * HH * WW
N_CORES = 8
INF = 16384.0
S_MAX = 8  # bf16-exact EDT bound (g <= 4*S^2 <= 256)
MAGIC = 0x7EF5  # bf16 reciprocal bit-hack constant

_nc_cache = {}
LAST_RESULTS = None  # test harness introspection


def _ensure_paths():
    for p in ("/opt/trn_rl_repo",):
        if os.path.isdir(p) and p not in sys.path:
            sys.path.insert(0, p)


def _bf16():
    import ml_dtypes

    return ml_dtypes.bfloat16


def _edt_sq_trunc_np(f0, S):
    """Truncated-shift separable squared EDT (numpy, int32); masks stacked on axis 0."""
    f = f0.astype(np.int32)
    nd = f.ndim
    for ax in (nd - 3, nd - 2, nd - 1):
        g = f.copy()
        for s in range(1, S + 1):
            s2 = s * s
            sl_out = [slice(None)] * nd
            sl_in = [slice(None)] * nd
            sl_out[ax] = slice(s, None)
            sl_in[ax] = slice(None, -s)
            np.minimum(g[tuple(sl_out)], f[tuple(sl_in)] + s2, out=g[tuple(sl_out)])
            sl_out[ax] = slice(None, -s)
            sl_in[ax] = slice(s, None)
            np.minimum(g[tuple(sl_out)], f[tuple(sl_in)] + s2, out=g[tuple(sl_out)])
        f = g
    return f


def _certified_shift_bound(masks):
    """Smallest S whose truncated EDT is provably exact for all masks: the exact
    nearest seed of any voxel v has per-axis offset <= sqrt(g_exact(v)) <=
    sqrt(max g_trunc), so floor(sqrt(max_g_trunc)) <= S puts it in the box."""
    stacked = np.stack(masks)  # (n, D, H, W)
    f0 = np.where(stacked, 0, 30000).astype(np.int32)
    for S in range(1, S_MAX + 1):
        g = _edt_sq_trunc_np(f0, S)
        if int(np.floor(np.sqrt(float(g.max())))) <= S:
            return S
    return S_MAX + 1  # triggers the fallback path


def _reference_fallback(pred, target):
    """Exact numpy replica of the reference for inputs the device path does not
    cover (wrong shapes, class filling the whole volume, S > S_MAX)."""
    BIG = 1e9
    pred = np.asarray(pred, np.float32)
    target = np.asarray(target)
    b_, c_ = pred.shape[0], pred.shape[1]
    n = np.arange(pred.shape[-1])

    def minplus(f):
        d2 = ((n[:, None] - n[None, :]) ** 2).astype(np.float32)
        return (f[..., None, :] + d2).min(axis=-1)

    def edt(src):
        f = np.where(src, 0.0, BIG).astype(np.float32)
        for ax in (-3, -2, -1):
            f = np.moveaxis(minplus(np.moveaxis(f, ax, -1)), -1, ax)
        return np.sqrt(f)

    e = np.exp(pred - pred.max(axis=1, keepdims=True))
    sm = e / e.sum(axis=1, keepdims=True)
    total = 0.0
    for b in range(b_):
        for c in range(c_):
            pos = target[b] == c
            if not pos.any():
                continue
            sdf = edt(pos) - edt(~pos)
            total += float(np.abs(sm[b, c] * sdf).sum(dtype=np.float64))
    return np.float32(total / pred.size)


def _build_nc(S):
    """Build + compile the SPMD Bass program for certified shift radius S."""
    _ensure_paths()
    import concourse.tile as tile
    from concourse import bacc, mybir

    bf = mybir.dt.bfloat16
    i16 = mybir.dt.int16
    f32 = mybir.dt.float32
    ALU = mybir.AluOpType
    ACT = mybir.ActivationFunctionType

    NR = 96  # pos rows [0,48) + neg rows [48,96)
    PITCH = 48 + 2 * ((S + 1) // 2)  # even, pad >= S, keeps H shifts 4B-aligned
    FD = HH * PITCH

    nc = bacc.Bacc("TRN2", target_bir_lowering=False, debug=False)

    f_d = [
        nc.dram_tensor(f"f{i}", [NR, FD], bf, kind="ExternalInput")
        for i in range(2 * S + 1)
    ]  # f0, then per s: +s-shift(+s^2), -s-shift(+s^2)
    p1_d = nc.dram_tensor("p1", [NR, FD], bf, kind="ExternalInput")
    p2_d = nc.dram_tensor("p2", [NR, FD], bf, kind="ExternalInput")
    stb_d = nc.dram_tensor("statblob", [NR, 144], bf, kind="ExternalInput")
    out_d = nc.dram_tensor("out", [48, 1], f32, kind="ExternalOutput")

    def chunks():
        n0 = 0
        while n0 < FD:
            nn = min(512, FD - n0)
            yield n0, nn
            n0 += nn

    with tile.TileContext(nc) as tc:
        with (
            tc.tile_pool(name="main", bufs=1) as pool,
            tc.tile_pool(name="psum", bufs=1, space="PSUM") as psp,
        ):
            # ---- input DMAs, split across the two HWDGE rings so the EDT
            # images (sync ring) and softmax inputs (scalar ring) load in
            # parallel; all contiguous host-prelaid layouts ----
            Fs = []
            for i, fd_ in enumerate(f_d):
                ft = pool.tile([NR, FD], bf, tag=f"f{i}")
                nc.sync.dma_start(ft[:], fd_[:])
                Fs.append(ft)
            P1 = pool.tile([NR, FD], bf, tag="p1")
            nc.scalar.dma_start(P1[:], p1_d[:])
            P2 = pool.tile([NR, FD], bf, tag="p2")
            nc.scalar.dma_start(P2[:], p2_d[:])
            STB = pool.tile([NR, 144], bf, tag="stb")
            nc.scalar.dma_start(STB[:], stb_d[:])
            ONE = pool.tile([1, 1], f32, tag="one")
            nc.gpsimd.memset(ONE[:], 1.0)

            # ---- softmax side, emitted early so ACT/PE overlap the EDT.
            # P1 rows: [my class | other1] x 48 d; P2: [other2 | other3]. ----
            P1e = pool.tile([NR, FD], bf, tag="p1e")
            nc.scalar.activation(P1e[:], P1[:], ACT.Exp)
            P2e = pool.tile([NR, FD], bf, tag="p2e")
            nc.scalar.activation(P2e[:], P2[:], ACT.Exp)
            PD = psp.tile([48, FD], f32, tag="acc")
            for n0, nn in chunks():
                nc.tensor.matmul(
                    PD[:, n0 : n0 + nn], STB[:, 0:48], P1e[:, n0 : n0 + nn],
                    start=True, stop=False,
                )
            for n0, nn in chunks():
                nc.tensor.matmul(
                    PD[:, n0 : n0 + nn], STB[:, 48:96], P2e[:, n0 : n0 + nn],
                    start=False, stop=True,
                )
            DN = pool.tile([48, FD], bf, tag="dn")
            nc.scalar.activation(DN[:], PD[:], ACT.Copy)

            # preload the sqrt ACT table off the critical path
            DUM = pool.tile([1, 1], f32, tag="dum")
            nc.scalar.activation(DUM[:], ONE[:], ACT.Sqrt)

            # ---- D pass: mins against host-prelaid shifted+biased images ----
            A = pool.tile([NR, FD], bf, tag="A")
            nc.vector.tensor_tensor(A[:], Fs[0][:], Fs[1][:], ALU.min)
            for ft in Fs[2:]:
                nc.vector.tensor_tensor(A[:], A[:], ft[:], ALU.min)

            # ---- H pass, in place on A (biased copies snapshot A first;
            # shifts are PITCH multiples: 4B-aligned, 2x mode) ----
            BHs = []
            for s in range(1, S + 1):
                BH = pool.tile([NR, FD], bf, tag=f"bh{s}")
                nc.vector.tensor_scalar(
                    out=BH[:], in0=A[:], scalar1=float(s * s), scalar2=0.0,
                    op0=ALU.add, op1=ALU.bypass,
                )
                BHs.append(BH)
            for s in range(1, S + 1):
                o = s * PITCH
                BH = BHs[s - 1]
                nc.vector.tensor_tensor(
                    A[:, o:FD], A[:, o:FD], BH[:, 0 : FD - o], ALU.min
                )
                nc.vector.tensor_tensor(
                    A[:, 0 : FD - o], A[:, 0 : FD - o], BH[:, o:FD], ALU.min
                )

            # ---- W pass into G (not in place: the odd-radius biased copy is
            # built on the scalar engine from A, and an in-place pass would
            # stall every DVE min on that cross-engine read of A). Odd radii:
            # copy written shifted by s so both DVE mins stay 4B-aligned 2x;
            # even radii: unshifted DVE 4x copy. Even radii min first so the
            # ACT copy has time to land. The -s direction's last s columns
            # would read the source's final s pad columns (INF), so skipping
            # them is exact for any s <= pad. ----
            BWs = {}
            for s in range(1, S + 1):
                s2 = float(s * s)
                BW = pool.tile([NR, FD], bf, tag=f"bw{s}")
                if s % 2 == 1:
                    nc.gpsimd.memset(BW[:, 0:s], INF)
                    nc.scalar.activation(
                        BW[:, s:FD], A[:, 0 : FD - s], ACT.Copy, bias=s2
                    )
                else:
                    nc.vector.tensor_scalar(
                        out=BW[:], in0=A[:], scalar1=s2, scalar2=0.0,
                        op0=ALU.add, op1=ALU.bypass,
                    )
                BWs[s] = BW
            G = pool.tile([NR, FD], bf, tag="G")
            nc.vector.tensor_copy(G[:], A[:])
            for s in sorted(range(1, S + 1), key=lambda x: x % 2):
                BW = BWs[s]
                if s % 2 == 1:
                    nc.vector.tensor_tensor(G[:], G[:], BW[:], ALU.min)
                    nc.vector.tensor_tensor(
                        G[:, 0 : FD - 2 * s], G[:, 0 : FD - 2 * s],
                        BW[:, 2 * s : FD], ALU.min,
                    )
                else:
                    nc.vector.tensor_tensor(
                        G[:, s:FD], G[:, s:FD], BW[:, 0 : FD - s], ALU.min
                    )
                    nc.vector.tensor_tensor(
                        G[:, 0 : FD - s], G[:, 0 : FD - s], BW[:, s:FD], ALU.min
                    )

            # ---- pos+neg pair-sum on PE, |sdf| = sqrt(g_pos + g_neg) on ACT ----
            PS = psp.tile([48, FD], f32, tag="acc")  # reuses PD's banks
            for n0, nn in chunks():
                nc.tensor.matmul(
                    PS[:, n0 : n0 + nn], STB[:, 96:144], G[:, n0 : n0 + nn],
                    start=True, stop=True,
                )
            SD = pool.tile([48, FD], bf, tag="sd")
            nc.scalar.activation(SD[:], PS[:], ACT.Sqrt)

            # ---- reciprocal of the softmax denominator: bf16 bit hack + one
            # Newton step (R = (D*r0 - 2)*r0 = -1/D approx) ----
            R0 = pool.tile([48, FD], bf, tag="r0")
            nc.vector.tensor_scalar(
                out=R0[:].bitcast(i16), in0=DN[:].bitcast(i16),
                scalar1=-1, scalar2=MAGIC, op0=ALU.mult, op1=ALU.add,
            )
            TN = pool.tile([48, FD], bf, tag="tn")
            nc.vector.tensor_tensor(TN[:], DN[:], R0[:], ALU.mult)
            nc.vector.tensor_scalar(
                out=TN[:], in0=TN[:], scalar1=-2.0, scalar2=0.0,
                op0=ALU.add, op1=ALU.bypass,
            )
            R = pool.tile([48, FD], bf, tag="r")
            nc.vector.tensor_tensor(R[:], TN[:], R0[:], ALU.mult)

            # ---- w = softmax weight of the core's class (negated); my class
            # occupies P1e rows [0:48) so one multiply covers all d ----
            W48 = pool.tile([48, FD], bf, tag="w")
            nc.vector.tensor_tensor(W48[:], P1e[0:48, :], R[:], ALU.mult)

            # ---- u = w * |sdf|, row-reduce (negated; host flips sign) ----
            U = pool.tile([48, FD], bf, tag="u")
            AC = pool.tile([48, 1], f32, tag="ac")
            nc.vector.tensor_tensor(U[:], W48[:], SD[:], ALU.mult)
            nc.vector.reduce_sum(AC[:], U[:], axis=mybir.AxisListType.X)
            nc.sync.dma_start(out_d[:], AC[:])

    nc.compile()
    return nc


def _host_inputs(pred, tgt, S):
    """Per-core input arrays (host prep: shard, pad, sentinel, bf16 convert)."""
    bf16 = _bf16()
    PITCH = 48 + 2 * ((S + 1) // 2)
    FD = HH * PITCH

    # statblob cols: [0:48) class-sum for P1e, [48:96) for P2e, [96:144) the
    # pos+neg pair-sum matrix.
    statblob = np.zeros((96, 144), bf16)
    for half in range(2):
        statblob[half * 48 + np.arange(48), np.arange(48)] = 1
        statblob[half * 48 + np.arange(48), 48 + np.arange(48)] = 1
        statblob[half * 48 + np.arange(48), 96 + np.arange(48)] = 1

    in_maps = []
    for k in range(N_CORES):
        b, c = divmod(k, C)
        pos = tgt[b] == c  # (D, H, W)

        # f0 and its d-shifted +s^2-biased variants, pos rows then neg rows,
        # W padded to PITCH with INF.
        fpad = np.full((96, HH, PITCH), INF, np.float32)
        fpad[0:48, :, 0:WW] = np.where(pos, 0.0, INF)
        fpad[48:96, :, 0:WW] = np.where(pos, INF, 0.0)
        imgs = [fpad]
        for s in range(1, S + 1):
            for sgn in (1, -1):
                im = np.full((96, HH, PITCH), INF, np.float32)
                if sgn > 0:
                    im[s:48] = fpad[0 : 48 - s] + s * s
                    im[48 + s : 96] = fpad[48 : 96 - s] + s * s
                else:
                    im[0 : 48 - s] = fpad[s:48] + s * s
                    im[48 : 96 - s] = fpad[48 + s : 96] + s * s
                np.minimum(im, INF, out=im)  # keep INF+s^2 clamped
                imgs.append(im)

        # exp-input layouts: P1 rows = [my class | other1] x 48 d, P2 rows =
        # [other2 | other3]; free = (h, w padded). Pad cols -80 (-> e ~ 0)
        # for my class, 0 (-> e = 1) elsewhere keep the denominator pad
        # finite and the weighted sum's pad contribution ~0.
        others = [j for j in range(C) if j != c]
        P1 = np.zeros((96, HH, PITCH), np.float32)
        P2 = np.zeros((96, HH, PITCH), np.float32)
        P1[0:48, :, 0:WW] = pred[b, c]
        P1[48:96, :, 0:WW] = pred[b, others[0]]
        P2[0:48, :, 0:WW] = pred[b, others[1]]
        P2[48:96, :, 0:WW] = pred[b, others[2]]
        P1[0:48, :, WW:] = -80.0

        m = {
            "p1": P1.reshape(96, FD).astype(bf16),
            "p2": P2.reshape(96, FD).astype(bf16),
            "statblob": statblob,
        }
        for i, im in enumerate(imgs):
            m[f"f{i}"] = im.reshape(96, FD).astype(bf16)
        in_maps.append(m)
    return in_maps


def kernel(pred, target):
    pred = np.ascontiguousarray(np.asarray(pred), dtype=np.float32)
    target = np.asarray(target)

    if pred.shape != (B, C, DD, HH, WW) or target.shape != (B, DD, HH, WW):
        return _reference_fallback(pred, target)

    tgt = target.astype(np.int64)
    masks = []
    has_pos = {}
    for b in range(B):
        for c in range(C):
            m = tgt[b] == c
            has_pos[(b, c)] = bool(m.any())
            if has_pos[(b, c)]:
                mn = ~m
                if not mn.any():
                    return _reference_fallback(pred, target)  # class fills volume
                masks.append(m)
                masks.append(mn)

    S = _certified_shift_bound(masks)
    if S > S_MAX:
        return _reference_fallback(pred, target)

    _ensure_paths()
    from concourse.bass_utils import run_bass_kernel_spmd

    if S not in _nc_cache:
        _nc_cache[S] = _build_nc(S)
    nc = _nc_cache[S]

    in_maps = _host_inputs(pred, tgt, S)

    trace = bool(os.environ.get("BOUNDARY_KERNEL_TRACE"))
    if trace:
        import importlib.util

        if importlib.util.find_spec("antenv.axon_hooks") is None:
            trace = False  # NTFF hook unavailable in this axon build
    res = run_bass_kernel_spmd(nc, in_maps, list(range(N_CORES)), trace=trace)
    global LAST_RESULTS
    LAST_RESULTS = res

    total = 0.0
    for k in range(N_CORES):
        b, c = divmod(k, C)
        if has_pos[(b, c)]:
            # device partials carry the Newton-reciprocal sign flip
            total -= float(res.results[k]["out"].astype(np.float64).sum())
    return np.float32(total / (B * C * NVOX))


if __name__ == "__main__":
    import reference

    inputs = reference.setup_inputs()
    out = kernel(**{k: np.asarray(v) for k, v in inputs.items()})
    print("kernel out:", out)


# revision 20
# speedup vs baseline: 1.0845x; 1.0845x over previous
"""Trainium2 Bass kernel for nn_BoundaryLoss: mean(|softmax(pred) * SDF(onehot(target))|).

Strategy (8 NeuronCores, SPMD, one (b, c) pair per core):
  - Exact 3D squared EDT of the class-c mask (pos) and complement (neg) via
    truncated-shift separable min-plus with certified radius S (host proves the
    truncation exact: if the S-truncated result's max squared distance M satisfies
    floor(sqrt(M)) <= S, every voxel's optimal seed lies in the [-S,S]^3 box).
  - All EDT arithmetic in bf16 (exact: values are small integers or INF=16384,
    which only meets min/add ops that keep it >= any real distance).
  - Layout [96 partitions, 48*PITCH free]: rows [0,48) pos | [48,96) neg;
    free = (h, w padded to PITCH with INF). The D (partition) pass consumes
    host-prelaid shifted+biased f0 images streamed from DRAM, so it is pure
    2x-mode tensor_tensor mins with no partition-offset DMA. H and W passes
    build one biased copy per radius with a 4x tensor_scalar (written shifted
    for odd W radii so every min stays 4B-aligned / 2x-mode) and min with
    plain tensor_tensor (the fused scalar_tensor_tensor only has a 1x uop).
  - softmax: ACT exp on host-prelaid (class,d)-partition layouts; denominator
    via PE matmul with a class-sum stationary; reciprocal via bf16 bit-hack +
    one Newton step on DVE (bass bans the ACT Reciprocal table; DVE reciprocal
    is ~6 cyc/elem). pos+neg pair-sum via PE matmul, sqrt on ACT from PSUM,
    final row-reduce on ACT via activation accum_out (scale=-1 absorbs the
    Newton sign).
  - Host shards inputs, sums the 8x48 f32 partials, applies the has_pos gate
    and the 1/(B*C*D*H*W) factor.
"""

import os
import sys

import numpy as np

B, C, DD, HH, WW = 2, 4, 48, 48, 48
NVOX = DD * HH * WW
N_CORES = 8
INF = 16384.0
S_MAX = 8  # bf16-exact EDT bound (g <= 4*S^2 <= 256)
MAGIC = 0x7EF5  # bf16 reciprocal bit-hack constant

_nc_cache = {}
LAST_RESULTS = None  # test harness introspection


def _ensure_paths():
    for p in ("/opt/trn_rl_repo",):
        if os.path.isdir(p) and p not in sys.path:
            sys.path.insert(0, p)


def _bf16():
    import ml_dtypes

    return ml_dtypes.bfloat16


def _edt_sq_trunc_np(f0, S):
    """Truncated-shift separable squared EDT (numpy, int32); masks stacked on axis 0."""
    f = f0.astype(np.int32)
    nd = f.ndim
    for ax in (nd - 3, nd - 2, nd - 1):
        g = f.copy()
        for s in range(1, S + 1):
            s2 = s * s
            sl_out = [slice(None)] * nd
            sl_in = [slice(None)] * nd
            sl_out[ax] = slice(s, None)
            sl_in[ax] = slice(None, -s)
            np.minimum(g[tuple(sl_out)], f[tuple(sl_in)] + s2, out=g[tuple(sl_out)])
            sl_out[ax] = slice(None, -s)
            sl_in[ax] = slice(s, None)
            np.minimum(g[tuple(sl_out)], f[tuple(sl_in)] + s2, out=g[tuple(sl_out)])
        f = g
    return f


def _certified_shift_bound(masks):
    """Smallest S whose truncated EDT is provably exact for all masks: the exact
    nearest seed of any voxel v has per-axis offset <= sqrt(g_exact(v)) <=
    sqrt(max g_trunc), so floor(sqrt(max_g_trunc)) <= S puts it in the box."""
    stacked = np.stack(masks)  # (n, D, H, W)
    f0 = np.where(stacked, 0, 30000).astype(np.int32)
    for S in range(1, S_MAX + 1):
        g = _edt_sq_trunc_np(f0, S)
        if int(np.floor(np.sqrt(float(g.max())))) <= S:
            return S
    return S_MAX + 1  # triggers the fallback path


def _reference_fallback(pred, target):
    """Exact numpy replica of the reference for inputs the device path does not
    cover (wrong shapes, class filling the whole volume, S > S_MAX)."""
    BIG = 1e9
    pred = np.asarray(pred, np.float32)
    target = np.asarray(target)
    b_, c_ = pred.shape[0], pred.shape[1]
    n = np.arange(pred.shape[-1])

    def minplus(f):
        d2 = ((n[:, None] - n[None, :]) ** 2).astype(np.float32)
        return (f[..., None, :] + d2).min(axis=-1)

    def edt(src):
        f = np.where(src, 0.0, BIG).astype(np.float32)
        for ax in (-3, -2, -1):
            f = np.moveaxis(minplus(np.moveaxis(f, ax, -1)), -1, ax)
        return np.sqrt(f)

    e = np.exp(pred - pred.max(axis=1, keepdims=True))
    sm = e / e.sum(axis=1, keepdims=True)
    total = 0.0
    for b in range(b_):
        for c in range(c_):
            pos = target[b] == c
            if not pos.any():
                continue
            sdf = edt(pos) - edt(~pos)
            total += float(np.abs(sm[b, c] * sdf).sum(dtype=np.float64))
    return np.float32(total / pred.size)


def _build_nc(S):
    """Build + compile the SPMD Bass program for certified shift radius S."""
    _ensure_paths()
    import concourse.tile as tile
    from concourse import bacc, mybir

    bf = mybir.dt.bfloat16
    i16 = mybir.dt.int16
    f32 = mybir.dt.float32
    ALU = mybir.AluOpType
    ACT = mybir.ActivationFunctionType

    NR = 96  # pos rows [0,48) + neg rows [48,96)
    PITCH = 48 + 2 * ((S + 1) // 2)  # even, pad >= S, keeps H shifts 4B-aligned
    FD = HH * PITCH

    nc = bacc.Bacc("TRN2", target_bir_lowering=False, debug=False)

    f_d = [
        nc.dram_tensor(f"f{i}", [NR, FD], bf, kind="ExternalInput")
        for i in range(2 * S + 1)
    ]  # f0, then per s: +s-shift(+s^2), -s-shift(+s^2)
    p1_d = nc.dram_tensor("p1", [NR, FD], bf, kind="ExternalInput")
    p2_d = nc.dram_tensor("p2", [NR, FD], bf, kind="ExternalInput")
    stb_d = nc.dram_tensor("statblob", [NR, 144], bf, kind="ExternalInput")
    out_d = nc.dram_tensor("out", [48, 2], f32, kind="ExternalOutput")

    def chunks():
        n0 = 0
        while n0 < FD:
            nn = min(512, FD - n0)
            yield n0, nn
            n0 += nn

    with tile.TileContext(nc) as tc:
        with (
            tc.tile_pool(name="main", bufs=1) as pool,
            tc.tile_pool(name="psum", bufs=1, space="PSUM") as psp,
        ):
            # ---- input DMAs, split across the two HWDGE rings so the EDT
            # images (sync ring) and softmax inputs (scalar ring) load in
            # parallel; all contiguous host-prelaid layouts ----
            Fs = []
            for i, fd_ in enumerate(f_d):
                ft = pool.tile([NR, FD], bf, tag=f"f{i}")
                nc.sync.dma_start(ft[:], fd_[:])
                Fs.append(ft)
            P1 = pool.tile([NR, FD], bf, tag="p1")
            nc.scalar.dma_start(P1[:], p1_d[:])
            P2 = pool.tile([NR, FD], bf, tag="p2")
            nc.scalar.dma_start(P2[:], p2_d[:])
            STB = pool.tile([NR, 144], bf, tag="stb")
            nc.scalar.dma_start(STB[:], stb_d[:])
            ONE = pool.tile([1, 1], f32, tag="one")
            nc.gpsimd.memset(ONE[:], 1.0)

            # ---- D pass: mins against host-prelaid shifted+biased images.
            # Emitted before the softmax section so the scheduler starts the
            # DVE chain as soon as the first two images land. ----
            A = pool.tile([NR, FD], bf, tag="A")
            nc.vector.tensor_tensor(A[:], Fs[0][:], Fs[1][:], ALU.min)
            for ft in Fs[2:]:
                nc.vector.tensor_tensor(A[:], A[:], ft[:], ALU.min)

            # ---- H pass, in place on A (biased copies snapshot A first;
            # shifts are PITCH multiples: 4B-aligned, 2x mode) ----
            BHs = []
            for s in range(1, S + 1):
                BH = pool.tile([NR, FD], bf, tag=f"bh{s}")
                nc.vector.tensor_scalar(
                    out=BH[:], in0=A[:], scalar1=float(s * s), scalar2=0.0,
                    op0=ALU.add, op1=ALU.bypass,
                )
                BHs.append(BH)
            for s in range(1, S + 1):
                o = s * PITCH
                BH = BHs[s - 1]
                nc.vector.tensor_tensor(
                    A[:, o:FD], A[:, o:FD], BH[:, 0 : FD - o], ALU.min
                )
                nc.vector.tensor_tensor(
                    A[:, 0 : FD - o], A[:, 0 : FD - o], BH[:, o:FD], ALU.min
                )

            # ---- softmax side on ACT/PE, overlapping the DVE EDT chain.
            # P1 rows: [my class | other1] x 48 d; P2: [other2 | other3]. ----
            P1e = pool.tile([NR, FD], bf, tag="p1e")
            nc.scalar.activation(P1e[:], P1[:], ACT.Exp)
            P2e = pool.tile([NR, FD], bf, tag="p2e")
            nc.scalar.activation(P2e[:], P2[:], ACT.Exp)
            PD = psp.tile([48, FD], f32, tag="acc")
            for n0, nn in chunks():
                nc.tensor.matmul(
                    PD[:, n0 : n0 + nn], STB[:, 0:48], P1e[:, n0 : n0 + nn],
                    start=True, stop=False,
                )
            for n0, nn in chunks():
                nc.tensor.matmul(
                    PD[:, n0 : n0 + nn], STB[:, 48:96], P2e[:, n0 : n0 + nn],
                    start=False, stop=True,
                )
            DN = pool.tile([48, FD], bf, tag="dn")
            nc.scalar.activation(DN[:], PD[:], ACT.Copy)

            # preload the sqrt ACT table off the critical path
            DUM = pool.tile([1, 1], f32, tag="dum")
            nc.scalar.activation(DUM[:], ONE[:], ACT.Sqrt)

            # ---- reciprocal of the softmax denominator: bf16 bit hack + one
            # Newton step (R = (D*r0 - 2)*r0 = -1/D approx). Runs on DVE
            # between the H and W passes so the post-W tail is only the
            # pair-sum / sqrt / weighted-reduce pipeline. ----
            R0 = pool.tile([48, FD], bf, tag="r0")
            nc.vector.tensor_scalar(
                out=R0[:].bitcast(i16), in0=DN[:].bitcast(i16),
                scalar1=-1, scalar2=MAGIC, op0=ALU.mult, op1=ALU.add,
            )
            TN = pool.tile([48, FD], bf, tag="tn")
            nc.vector.tensor_tensor(TN[:], DN[:], R0[:], ALU.mult)
            nc.vector.tensor_scalar(
                out=TN[:], in0=TN[:], scalar1=-2.0, scalar2=0.0,
                op0=ALU.add, op1=ALU.bypass,
            )
            R = pool.tile([48, FD], bf, tag="r")
            nc.vector.tensor_tensor(R[:], TN[:], R0[:], ALU.mult)
            W48 = pool.tile([48, FD], bf, tag="w")
            nc.vector.tensor_tensor(W48[:], P1e[0:48, :], R[:], ALU.mult)

            # ---- W pass into G (not in place: the odd-radius biased copy is
            # built on the scalar engine from A, and an in-place pass would
            # stall every DVE min on that cross-engine read of A). Odd radii:
            # copy written shifted by s so both DVE mins stay 4B-aligned 2x;
            # even radii: unshifted DVE 4x copy. Even radii min first so the
            # ACT copy has time to land. The -s direction's last s columns
            # would read the source's final s pad columns (INF), so skipping
            # them is exact for any s <= pad. ----
            BWs = {}
            for s in range(1, S + 1):
                s2 = float(s * s)
                BW = pool.tile([NR, FD], bf, tag=f"bw{s}")
                if s % 2 == 1:
                    nc.gpsimd.memset(BW[:, 0:s], INF)
                    nc.scalar.activation(
                        BW[:, s:FD], A[:, 0 : FD - s], ACT.Copy, bias=s2
                    )
                else:
                    nc.vector.tensor_scalar(
                        out=BW[:], in0=A[:], scalar1=s2, scalar2=0.0,
                        op0=ALU.add, op1=ALU.bypass,
                    )
                BWs[s] = BW
            G = pool.tile([NR, FD], bf, tag="G")
            nc.vector.tensor_copy(G[:], A[:])
            for s in sorted(range(1, S + 1), key=lambda x: x % 2):
                BW = BWs[s]
                if s % 2 == 1:
                    nc.vector.tensor_tensor(G[:], G[:], BW[:], ALU.min)
                    nc.vector.tensor_tensor(
                        G[:, 0 : FD - 2 * s], G[:, 0 : FD - 2 * s],
                        BW[:, 2 * s : FD], ALU.min,
                    )
                else:
                    nc.vector.tensor_tensor(
                        G[:, s:FD], G[:, s:FD], BW[:, 0 : FD - s], ALU.min
                    )
                    nc.vector.tensor_tensor(
                        G[:, 0 : FD - s], G[:, 0 : FD - s], BW[:, s:FD], ALU.min
                    )

            # ---- tail, pipelined in two free-halves: pos+neg pair-sum on PE,
            # |sdf| = sqrt(g_pos + g_neg) on ACT from PSUM, u = w * |sdf| and
            # row-reduce on DVE (negated; host flips sign) ----
            PS = psp.tile([48, FD], f32, tag="acc")  # reuses PD's banks
            SD = pool.tile([48, FD], bf, tag="sd")
            U = pool.tile([48, FD], bf, tag="u")
            AC = pool.tile([48, 2], f32, tag="ac")
            half = (FD // 2 + 511) // 512 * 512  # bank-aligned split
            for hi, (h0, h1) in enumerate(((0, half), (half, FD))):
                n0 = h0
                while n0 < h1:
                    nn = min(512, h1 - n0)
                    nc.tensor.matmul(
                        PS[:, n0 : n0 + nn], STB[:, 96:144], G[:, n0 : n0 + nn],
                        start=True, stop=True,
                    )
                    n0 += nn
                nc.scalar.activation(SD[:, h0:h1], PS[:, h0:h1], ACT.Sqrt)
                nc.vector.tensor_tensor(
                    U[:, h0:h1], W48[:, h0:h1], SD[:, h0:h1], ALU.mult
                )
                nc.vector.reduce_sum(
                    AC[:, hi : hi + 1], U[:, h0:h1], axis=mybir.AxisListType.X
                )
            nc.sync.dma_start(out_d[:], AC[:])

    nc.compile()
    return nc


def _host_inputs(pred, tgt, S):
    """Per-core input arrays (host prep: shard, pad, sentinel, bf16 convert)."""
    bf16 = _bf16()
    PITCH = 48 + 2 * ((S + 1) // 2)
    FD = HH * PITCH

    # statblob cols: [0:48) class-sum for P1e, [48:96) for P2e, [96:144) the
    # pos+neg pair-sum matrix.
    statblob = np.zeros((96, 144), bf16)
    for half in range(2):
        statblob[half * 48 + np.arange(48), np.arange(48)] = 1
        statblob[half * 48 + np.arange(48), 48 + np.arange(48)] = 1
        statblob[half * 48 + np.arange(48), 96 + np.arange(48)] = 1

    in_maps = []
    for k in range(N_CORES):
        b, c = divmod(k, C)
        pos = tgt[b] == c  # (D, H, W)

        # f0 and its d-shifted +s^2-biased variants, pos rows then neg rows,
        # W padded to PITCH with INF.
        fpad = np.full((96, HH, PITCH), INF, np.float32)
        fpad[0:48, :, 0:WW] = np.where(pos, 0.0, INF)
        fpad[48:96, :, 0:WW] = np.where(pos, INF, 0.0)
        imgs = [fpad]
        for s in range(1, S + 1):
            for sgn in (1, -1):
                im = np.full((96, HH, PITCH), INF, np.float32)
                if sgn > 0:
                    im[s:48] = fpad[0 : 48 - s] + s * s
                    im[48 + s : 96] = fpad[48 : 96 - s] + s * s
                else:
                    im[0 : 48 - s] = fpad[s:48] + s * s
                    im[48 : 96 - s] = fpad[48 + s : 96] + s * s
                np.minimum(im, INF, out=im)  # keep INF+s^2 clamped
                imgs.append(im)

        # exp-input layouts: P1 rows = [my class | other1] x 48 d, P2 rows =
        # [other2 | other3]; free = (h, w padded). Pad cols -80 (-> e ~ 0)
        # for my class, 0 (-> e = 1) elsewhere keep the denominator pad
        # finite and the weighted sum's pad contribution ~0.
        others = [j for j in range(C) if j != c]
        P1 = np.zeros((96, HH, PITCH), np.float32)
        P2 = np.zeros((96, HH, PITCH), np.float32)
        P1[0:48, :, 0:WW] = pred[b, c]
        P1[48:96, :, 0:WW] = pred[b, others[0]]
        P2[0:48, :, 0:WW] = pred[b, others[1]]
        P2[48:96, :, 0:WW] = pred[b, others[2]]
        P1[0:48, :, WW:] = -80.0

        m = {
            "p1": P1.reshape(96, FD).astype(bf16),
            "p2": P2.reshape(96, FD).astype(bf16),
            "statblob": statblob,
        }
        for i, im in enumerate(imgs):
            m[f"f{i}"] = im.reshape(96, FD).astype(bf16)
        in_maps.append(m)
    return in_maps


def kernel(pred, target):
    pred = np.ascontiguousarray(np.asarray(pred), dtype=np.float32)
    target = np.asarray(target)

    if pred.shape != (B, C, DD, HH, WW) or target.shape != (B, DD, HH, WW):
        return _reference_fallback(pred, target)

    tgt = target.astype(np.int64)
    masks = []
    has_pos = {}
    for b in range(B):
        for c in range(C):
            m = tgt[b] == c
            has_pos[(b, c)] = bool(m.any())
            if has_pos[(b, c)]:
                mn = ~m
                if not mn.any():
                    return _reference_fallback(pred, target)  # class fills volume
                masks.append(m)
                masks.append(mn)

    S = _certified_shift_bound(masks)
    if S > S_MAX:
        return _reference_fallback(pred, target)

    _ensure_paths()
    from concourse.bass_utils import run_bass_kernel_spmd

    if S not in _nc_cache:
        _nc_cache[S] = _build_nc(S)
    nc = _nc_cache[S]

    in_maps = _host_inputs(pred, tgt, S)

    trace = bool(os.environ.get("BOUNDARY_KERNEL_TRACE"))
    if trace:
        import importlib.util

        if importlib.util.find_spec("antenv.axon_hooks") is None:
            trace = False  # NTFF hook unavailable in this axon build
    res = run_bass_kernel_spmd(nc, in_maps, list(range(N_CORES)), trace=trace)
    global LAST_RESULTS
    LAST_RESULTS = res

    total = 0.0
    for k in range(N_CORES):
        b, c = divmod(k, C)
        if has_pos[(b, c)]:
            # device partials carry the Newton-reciprocal sign flip
            total -= float(res.results[k]["out"].astype(np.float64).sum())
    return np.float32(total / (B * C * NVOX))


if __name__ == "__main__":
    import reference

    inputs = reference.setup_inputs()
    out = kernel(**{k: np.asarray(v) for k, v in inputs.items()})
    print("kernel out:", out)


# revision 23
# speedup vs baseline: 1.0990x; 1.0133x over previous
"""Trainium2 Bass kernel for nn_BoundaryLoss: mean(|softmax(pred) * SDF(onehot(target))|).

Strategy (8 NeuronCores, SPMD, one (b, c) pair per core):
  - Exact 3D squared EDT of the class-c mask (pos) and complement (neg) via
    truncated-shift separable min-plus with certified radius S (host proves the
    truncation exact: if the S-truncated result's max squared distance M satisfies
    floor(sqrt(M)) <= S, every voxel's optimal seed lies in the [-S,S]^3 box).
  - All EDT arithmetic in bf16 (exact: values are small integers or INF=16384,
    which only meets min/add ops that keep it >= any real distance).
  - Layout [96 partitions, 48*PITCH free]: rows [0,48) pos | [48,96) neg;
    free = (h, w padded to PITCH with INF). The D (partition) pass consumes
    host-prelaid shifted+biased f0 images streamed from DRAM, so it is pure
    2x-mode tensor_tensor mins with no partition-offset DMA. H and W passes
    build one biased copy per radius with a 4x tensor_scalar (written shifted
    for odd W radii so every min stays 4B-aligned / 2x-mode) and min with
    plain tensor_tensor (the fused scalar_tensor_tensor only has a 1x uop).
  - softmax: ACT exp on host-prelaid (class,d)-partition layouts; denominator
    via PE matmul with a class-sum stationary; reciprocal via bf16 bit-hack +
    one Newton step on DVE (bass bans the ACT Reciprocal table; DVE reciprocal
    is ~6 cyc/elem). pos+neg pair-sum via PE matmul, sqrt on ACT from PSUM,
    final row-reduce on ACT via activation accum_out (scale=-1 absorbs the
    Newton sign).
  - Host shards inputs, sums the 8x48 f32 partials, applies the has_pos gate
    and the 1/(B*C*D*H*W) factor.
"""

import os
import sys

import numpy as np

B, C, DD, HH, WW = 2, 4, 48, 48, 48
NVOX = DD * HH * WW
N_CORES = 8
INF = 16384.0
S_MAX = 8  # bf16-exact EDT bound (g <= 4*S^2 <= 256)
MAGIC = 0x7EF5  # bf16 reciprocal bit-hack constant

_nc_cache = {}
LAST_RESULTS = None  # test harness introspection


def _ensure_paths():
    for p in ("/opt/trn_rl_repo",):
        if os.path.isdir(p) and p not in sys.path:
            sys.path.insert(0, p)


def _bf16():
    import ml_dtypes

    return ml_dtypes.bfloat16


def _edt_sq_trunc_np(f0, S):
    """Truncated-shift separable squared EDT (numpy, int32); masks stacked on axis 0."""
    f = f0.astype(np.int32)
    nd = f.ndim
    for ax in (nd - 3, nd - 2, nd - 1):
        g = f.copy()
        for s in range(1, S + 1):
            s2 = s * s
            sl_out = [slice(None)] * nd
            sl_in = [slice(None)] * nd
            sl_out[ax] = slice(s, None)
            sl_in[ax] = slice(None, -s)
            np.minimum(g[tuple(sl_out)], f[tuple(sl_in)] + s2, out=g[tuple(sl_out)])
            sl_out[ax] = slice(None, -s)
            sl_in[ax] = slice(s, None)
            np.minimum(g[tuple(sl_out)], f[tuple(sl_in)] + s2, out=g[tuple(sl_out)])
        f = g
    return f


def _certified_shift_bound(masks):
    """Smallest S whose truncated EDT is provably exact for all masks: the exact
    nearest seed of any voxel v has per-axis offset <= sqrt(g_exact(v)) <=
    sqrt(max g_trunc), so floor(sqrt(max_g_trunc)) <= S puts it in the box."""
    stacked = np.stack(masks)  # (n, D, H, W)
    f0 = np.where(stacked, 0, 30000).astype(np.int32)
    for S in range(1, S_MAX + 1):
        g = _edt_sq_trunc_np(f0, S)
        if int(np.floor(np.sqrt(float(g.max())))) <= S:
            return S
    return S_MAX + 1  # triggers the fallback path


def _reference_fallback(pred, target):
    """Exact numpy replica of the reference for inputs the device path does not
    cover (wrong shapes, class filling the whole volume, S > S_MAX)."""
    BIG = 1e9
    pred = np.asarray(pred, np.float32)
    target = np.asarray(target)
    b_, c_ = pred.shape[0], pred.shape[1]
    n = np.arange(pred.shape[-1])

    def minplus(f):
        d2 = ((n[:, None] - n[None, :]) ** 2).astype(np.float32)
        return (f[..., None, :] + d2).min(axis=-1)

    def edt(src):
        f = np.where(src, 0.0, BIG).astype(np.float32)
        for ax in (-3, -2, -1):
            f = np.moveaxis(minplus(np.moveaxis(f, ax, -1)), -1, ax)
        return np.sqrt(f)

    e = np.exp(pred - pred.max(axis=1, keepdims=True))
    sm = e / e.sum(axis=1, keepdims=True)
    total = 0.0
    for b in range(b_):
        for c in range(c_):
            pos = target[b] == c
            if not pos.any():
                continue
            sdf = edt(pos) - edt(~pos)
            total += float(np.abs(sm[b, c] * sdf).sum(dtype=np.float64))
    return np.float32(total / pred.size)


def _build_nc(S):
    """Build + compile the SPMD Bass program for certified shift radius S."""
    _ensure_paths()
    import concourse.tile as tile
    from concourse import bacc, mybir

    bf = mybir.dt.bfloat16
    i16 = mybir.dt.int16
    f32 = mybir.dt.float32
    ALU = mybir.AluOpType
    ACT = mybir.ActivationFunctionType

    NR = 96  # pos rows [0,48) + neg rows [48,96)
    PITCH = 48 + 2 * ((S + 1) // 2)  # even, pad >= S, keeps H shifts 4B-aligned
    FD = HH * PITCH

    nc = bacc.Bacc("TRN2", target_bir_lowering=False, debug=False)

    f_d = [
        nc.dram_tensor(f"f{i}", [NR, FD], bf, kind="ExternalInput")
        for i in range(2 * S + 1)
    ]  # f0, then per s: +s-shift(+s^2), -s-shift(+s^2)
    p1_d = nc.dram_tensor("p1", [NR, FD], bf, kind="ExternalInput")
    p2_d = nc.dram_tensor("p2", [NR, FD], bf, kind="ExternalInput")
    stb_d = nc.dram_tensor("statblob", [NR, 144], bf, kind="ExternalInput")
    n_tail = 4 if FD > 2048 else 2
    out_d = nc.dram_tensor("out", [48, n_tail], f32, kind="ExternalOutput")

    def chunks():
        n0 = 0
        while n0 < FD:
            nn = min(512, FD - n0)
            yield n0, nn
            n0 += nn

    with tile.TileContext(nc) as tc:
        with (
            tc.tile_pool(name="main", bufs=1) as pool,
            tc.tile_pool(name="psum", bufs=1, space="PSUM") as psp,
        ):
            # ---- input DMAs, split across the two HWDGE rings so the EDT
            # images (sync ring) and softmax inputs (scalar ring) load in
            # parallel; all contiguous host-prelaid layouts ----
            # Engines wake ~4-5us after their whole DMA ring completes, so the
            # sync ring keeps only the first three EDT images (gating the DVE
            # chain) and everything else rides the scalar ring.
            Fs = []
            for i, fd_ in enumerate(f_d):
                ft = pool.tile([NR, FD], bf, tag=f"f{i}")
                if i < 3:
                    nc.sync.dma_start(ft[:], fd_[:])
                Fs.append(ft)
            P1 = pool.tile([NR, FD], bf, tag="p1")
            nc.scalar.dma_start(P1[:], p1_d[:])
            P2 = pool.tile([NR, FD], bf, tag="p2")
            nc.scalar.dma_start(P2[:], p2_d[:])
            STB = pool.tile([NR, 144], bf, tag="stb")
            nc.scalar.dma_start(STB[:], stb_d[:])
            for i, fd_ in enumerate(f_d):
                if i >= 3:
                    nc.scalar.dma_start(Fs[i][:], fd_[:])
            ONE = pool.tile([1, 1], f32, tag="one")
            nc.gpsimd.memset(ONE[:], 1.0)

            # ---- D pass: mins against host-prelaid shifted+biased images.
            # Emitted before the softmax section so the scheduler starts the
            # DVE chain as soon as the first two images land. ----
            A = pool.tile([NR, FD], bf, tag="A")
            nc.vector.tensor_tensor(A[:], Fs[0][:], Fs[1][:], ALU.min)
            for ft in Fs[2:]:
                nc.vector.tensor_tensor(A[:], A[:], ft[:], ALU.min)

            # ---- H pass, in place on A (biased copies snapshot A first;
            # shifts are PITCH multiples: 4B-aligned, 2x mode) ----
            BHs = []
            for s in range(1, S + 1):
                BH = pool.tile([NR, FD], bf, tag=f"bh{s}")
                nc.vector.tensor_scalar(
                    out=BH[:], in0=A[:], scalar1=float(s * s), scalar2=0.0,
                    op0=ALU.add, op1=ALU.bypass,
                )
                BHs.append(BH)
            for s in range(1, S + 1):
                o = s * PITCH
                BH = BHs[s - 1]
                nc.vector.tensor_tensor(
                    A[:, o:FD], A[:, o:FD], BH[:, 0 : FD - o], ALU.min
                )
                nc.vector.tensor_tensor(
                    A[:, 0 : FD - o], A[:, 0 : FD - o], BH[:, o:FD], ALU.min
                )

            # ---- softmax side on ACT/PE, overlapping the DVE EDT chain.
            # P1 rows: [my class | other1] x 48 d; P2: [other2 | other3]. ----
            P1e = pool.tile([NR, FD], bf, tag="p1e")
            nc.scalar.activation(P1e[:], P1[:], ACT.Exp)
            P2e = pool.tile([NR, FD], bf, tag="p2e")
            nc.scalar.activation(P2e[:], P2[:], ACT.Exp)
            PD = psp.tile([48, FD], f32, tag="acc")
            for n0, nn in chunks():
                nc.tensor.matmul(
                    PD[:, n0 : n0 + nn], STB[:, 0:48], P1e[:, n0 : n0 + nn],
                    start=True, stop=False,
                )
            for n0, nn in chunks():
                nc.tensor.matmul(
                    PD[:, n0 : n0 + nn], STB[:, 48:96], P2e[:, n0 : n0 + nn],
                    start=False, stop=True,
                )
            DN = pool.tile([48, FD], bf, tag="dn")
            nc.scalar.activation(DN[:], PD[:], ACT.Copy)

            # preload the sqrt ACT table off the critical path
            DUM = pool.tile([1, 1], f32, tag="dum")
            nc.scalar.activation(DUM[:], ONE[:], ACT.Sqrt)

            # ---- reciprocal of the softmax denominator: bf16 bit hack + one
            # Newton step (R = (D*r0 - 2)*r0 = -1/D approx). Runs on DVE
            # between the H and W passes so the post-W tail is only the
            # pair-sum / sqrt / weighted-reduce pipeline. ----
            R0 = pool.tile([48, FD], bf, tag="r0")
            nc.vector.tensor_scalar(
                out=R0[:].bitcast(i16), in0=DN[:].bitcast(i16),
                scalar1=-1, scalar2=MAGIC, op0=ALU.mult, op1=ALU.add,
            )
            TN = pool.tile([48, FD], bf, tag="tn")
            nc.vector.tensor_tensor(TN[:], DN[:], R0[:], ALU.mult)
            nc.vector.tensor_scalar(
                out=TN[:], in0=TN[:], scalar1=-2.0, scalar2=0.0,
                op0=ALU.add, op1=ALU.bypass,
            )
            R = pool.tile([48, FD], bf, tag="r")
            nc.vector.tensor_tensor(R[:], TN[:], R0[:], ALU.mult)
            W48 = pool.tile([48, FD], bf, tag="w")
            nc.vector.tensor_tensor(W48[:], P1e[0:48, :], R[:], ALU.mult)

            # ---- W pass into G (not in place: the odd-radius biased copy is
            # built on the scalar engine from A, and an in-place pass would
            # stall every DVE min on that cross-engine read of A). Odd radii:
            # copy written shifted by s so both DVE mins stay 4B-aligned 2x;
            # even radii: unshifted DVE 4x copy. Even radii min first so the
            # ACT copy has time to land. The -s direction's last s columns
            # would read the source's final s pad columns (INF), so skipping
            # them is exact for any s <= pad. ----
            BWs = {}
            for s in range(1, S + 1):
                s2 = float(s * s)
                BW = pool.tile([NR, FD], bf, tag=f"bw{s}")
                if s % 2 == 1:
                    nc.gpsimd.memset(BW[:, 0:s], INF)
                    nc.scalar.activation(
                        BW[:, s:FD], A[:, 0 : FD - s], ACT.Copy, bias=s2
                    )
                else:
                    nc.vector.tensor_scalar(
                        out=BW[:], in0=A[:], scalar1=s2, scalar2=0.0,
                        op0=ALU.add, op1=ALU.bypass,
                    )
                BWs[s] = BW
            G = pool.tile([NR, FD], bf, tag="G")
            nc.vector.tensor_copy(G[:], A[:])
            for s in sorted(range(1, S + 1), key=lambda x: x % 2):
                BW = BWs[s]
                if s % 2 == 1:
                    nc.vector.tensor_tensor(G[:], G[:], BW[:], ALU.min)
                    nc.vector.tensor_tensor(
                        G[:, 0 : FD - 2 * s], G[:, 0 : FD - 2 * s],
                        BW[:, 2 * s : FD], ALU.min,
                    )
                else:
                    nc.vector.tensor_tensor(
                        G[:, s:FD], G[:, s:FD], BW[:, 0 : FD - s], ALU.min
                    )
                    nc.vector.tensor_tensor(
                        G[:, 0 : FD - s], G[:, 0 : FD - s], BW[:, s:FD], ALU.min
                    )

            # ---- tail, pipelined in two free-halves: pos+neg pair-sum on PE,
            # |sdf| = sqrt(g_pos + g_neg) on ACT from PSUM, u = w * |sdf| and
            # row-reduce on DVE (negated; host flips sign) ----
            PS = psp.tile([48, FD], f32, tag="acc")  # reuses PD's banks
            SD = pool.tile([48, FD], bf, tag="sd")
            U = pool.tile([48, FD], bf, tag="u")
            bounds = [0, 1024, 1536, 2048, FD] if FD > 2048 else [0, 1024, FD]
            AC = pool.tile([48, len(bounds) - 1], f32, tag="ac")
            for hi in range(len(bounds) - 1):
                h0, h1 = bounds[hi], bounds[hi + 1]
                n0 = h0
                while n0 < h1:
                    nn = min(512, h1 - n0)
                    nc.tensor.matmul(
                        PS[:, n0 : n0 + nn], STB[:, 96:144], G[:, n0 : n0 + nn],
                        start=True, stop=True,
                    )
                    n0 += nn
                nc.scalar.activation(SD[:, h0:h1], PS[:, h0:h1], ACT.Sqrt)
                nc.vector.tensor_tensor(
                    U[:, h0:h1], W48[:, h0:h1], SD[:, h0:h1], ALU.mult
                )
                nc.vector.reduce_sum(
                    AC[:, hi : hi + 1], U[:, h0:h1], axis=mybir.AxisListType.X
                )
            nc.sync.dma_start(out_d[:], AC[:])

    nc.compile()
    return nc


def _host_inputs(pred, tgt, S):
    """Per-core input arrays (host prep: shard, pad, sentinel, bf16 convert)."""
    bf16 = _bf16()
    PITCH = 48 + 2 * ((S + 1) // 2)
    FD = HH * PITCH

    # statblob cols: [0:48) class-sum for P1e, [48:96) for P2e, [96:144) the
    # pos+neg pair-sum matrix.
    statblob = np.zeros((96, 144), bf16)
    for half in range(2):
        statblob[half * 48 + np.arange(48), np.arange(48)] = 1
        statblob[half * 48 + np.arange(48), 48 + np.arange(48)] = 1
        statblob[half * 48 + np.arange(48), 96 + np.arange(48)] = 1

    in_maps = []
    for k in range(N_CORES):
        b, c = divmod(k, C)
        pos = tgt[b] == c  # (D, H, W)

        # f0 and its d-shifted +s^2-biased variants, pos rows then neg rows,
        # W padded to PITCH with INF.
        fpad = np.full((96, HH, PITCH), INF, np.float32)
        fpad[0:48, :, 0:WW] = np.where(pos, 0.0, INF)
        fpad[48:96, :, 0:WW] = np.where(pos, INF, 0.0)
        imgs = [fpad]
        for s in range(1, S + 1):
            for sgn in (1, -1):
                im = np.full((96, HH, PITCH), INF, np.float32)
                if sgn > 0:
                    im[s:48] = fpad[0 : 48 - s] + s * s
                    im[48 + s : 96] = fpad[48 : 96 - s] + s * s
                else:
                    im[0 : 48 - s] = fpad[s:48] + s * s
                    im[48 : 96 - s] = fpad[48 + s : 96] + s * s
                np.minimum(im, INF, out=im)  # keep INF+s^2 clamped
                imgs.append(im)

        # exp-input layouts: P1 rows = [my class | other1] x 48 d, P2 rows =
        # [other2 | other3]; free = (h, w padded). Pad cols -80 (-> e ~ 0)
        # for my class, 0 (-> e = 1) elsewhere keep the denominator pad
        # finite and the weighted sum's pad contribution ~0.
        others = [j for j in range(C) if j != c]
        P1 = np.zeros((96, HH, PITCH), np.float32)
        P2 = np.zeros((96, HH, PITCH), np.float32)
        P1[0:48, :, 0:WW] = pred[b, c]
        P1[48:96, :, 0:WW] = pred[b, others[0]]
        P2[0:48, :, 0:WW] = pred[b, others[1]]
        P2[48:96, :, 0:WW] = pred[b, others[2]]
        P1[0:48, :, WW:] = -80.0

        m = {
            "p1": P1.reshape(96, FD).astype(bf16),
            "p2": P2.reshape(96, FD).astype(bf16),
            "statblob": statblob,
        }
        for i, im in enumerate(imgs):
            m[f"f{i}"] = im.reshape(96, FD).astype(bf16)
        in_maps.append(m)
    return in_maps


def kernel(pred, target):
    pred = np.ascontiguousarray(np.asarray(pred), dtype=np.float32)
    target = np.asarray(target)

    if pred.shape != (B, C, DD, HH, WW) or target.shape != (B, DD, HH, WW):
        return _reference_fallback(pred, target)

    tgt = target.astype(np.int64)
    masks = []
    has_pos = {}
    for b in range(B):
        for c in range(C):
            m = tgt[b] == c
            has_pos[(b, c)] = bool(m.any())
            if has_pos[(b, c)]:
                mn = ~m
                if not mn.any():
                    return _reference_fallback(pred, target)  # class fills volume
                masks.append(m)
                masks.append(mn)

    S = _certified_shift_bound(masks)
    if S > S_MAX:
        return _reference_fallback(pred, target)

    _ensure_paths()
    from concourse.bass_utils import run_bass_kernel_spmd

    if S not in _nc_cache:
        _nc_cache[S] = _build_nc(S)
    nc = _nc_cache[S]

    in_maps = _host_inputs(pred, tgt, S)

    trace = bool(os.environ.get("BOUNDARY_KERNEL_TRACE"))
    if trace:
        import importlib.util

        if importlib.util.find_spec("antenv.axon_hooks") is None:
            trace = False  # NTFF hook unavailable in this axon build
    res = run_bass_kernel_spmd(nc, in_maps, list(range(N_CORES)), trace=trace)
    global LAST_RESULTS
    LAST_RESULTS = res

    total = 0.0
    for k in range(N_CORES):
        b, c = divmod(k, C)
        if has_pos[(b, c)]:
            # device partials carry the Newton-reciprocal sign flip
            total -= float(res.results[k]["out"].astype(np.float64).sum())
    return np.float32(total / (B * C * NVOX))


if __name__ == "__main__":
    import reference

    inputs = reference.setup_inputs()
    out = kernel(**{k: np.asarray(v) for k, v in inputs.items()})
    print("kernel out:", out)
